# revision 29
# baseline (speedup 1.0000x reference)
"""Trainium2 Bass kernel for nn_NeuralRandomForest (soft decision forest).

Math restructuring (validated against the reference in numpy):
  * out[:, 1] == 1 - out[:, 0] exactly (2-class softmax leaves; leaf probs
    and tree weights each sum to 1) -> only class 0 computed on device.
  * Bottom-up soft-tree evaluation on node *values*:
        V_node = V_left + sigma_node * (V_right - V_left)
    with the deepest internal level affine in sigma: V = wA + wB * sigma.
    No leaf-probability products, no wide final contraction.
  * Split-order (bit-reversed-prefix) storage makes every level update a
    dense step-1 slice op -> DVE fp16 2x_1P mode.
  * Tiny-tensor work (masking, softmaxes, permutation, transposing x,
    fp16 casts) is done on the host.

Mapping (per core; batch sharded 8 ways, parameters replicated):
  PE   : logits z = x @ WmT          (two MMs per 128-row tile: 512+108)
  ACT  : sigma = sigmoid(z)          (PSUM -> SBUF fp16)
  DVE  : bottom-up value collapse    (fp16 2x ops) + per-tile tree-sum
  SP   : HWDGE DMAs (x chunks in, outputs out)

Raw-bass pipeline with manual semaphores (Tile's conservative multi-wait
emission exceeds the single sync-wait the MM ISA struct allows).
"""

import sys
import numpy as np

for _p in ("/opt/trn_rl_repo", "/root/.axon_site/_ro/trn_rl_repo"):
    if _p not in sys.path:
        sys.path.insert(0, _p)

DEPTH = 5
T = 20
F = 128
B = 131072
N_CORES = 8
BPC = B // N_CORES          # 16384 rows per core
P = 128
PT = BPC // P               # 128 ptiles per core
G = 8                       # ptiles per supertile
NST = PT // G               # 16 supertiles
W = T * 31                  # 620 logit columns
LVL_OFF = [0, 1, 3, 7, 15]  # level offset in nodes-per-tree units

_prog_cache = {}
_last_in_maps = None


def _bitrev(v, bits):
    r = 0
    for _ in range(bits):
        r = (r << 1) | (v & 1)
        v >>= 1
    return r


def _build_program(with_bias):
    import concourse.bass as bass
    from concourse import mybir

    f16 = mybir.dt.float16
    f32 = mybir.dt.float32

    nc = bass.Bass()

    xt = nc.declare_dram_parameter("xt", [P, BPC], f16, isOutput=False)
    wmt = nc.declare_dram_parameter("wmt", [P, W], f16, isOutput=False)
    wax = nc.declare_dram_parameter("wax", [P, G * 320], f16, isOutput=False)
    wbx = nc.declare_dram_parameter("wbx", [P, G * 320], f16, isOutput=False)
    if with_bias:
        brow = nc.declare_dram_parameter("brow", [P, W], f16, isOutput=False)
        ones = nc.declare_dram_parameter("ones", [1, P], f16, isOutput=False)
    ident = nc.declare_dram_parameter("ident", [P, P], f32, isOutput=False)
    out = nc.declare_dram_parameter("out", [2, BPC], f32, isOutput=True)

    XSLOTS = 3   # xt supertile slots
    SSLOTS = 2   # sigmoid-array supertile slots
    PSLOTS = 4   # psum ptile slots (4 x 2 banks)

    from contextlib import ExitStack

    with ExitStack() as stack:
        e = stack.enter_context
        wmt_s = e(nc.sbuf_tensor([P, W], f16))
        wax_s = e(nc.sbuf_tensor([P, G * 320], f16))
        wbx_s = e(nc.sbuf_tensor([P, G * 320], f16))
        brow_s = e(nc.sbuf_tensor([P, W], f16))
        ones_s = e(nc.sbuf_tensor([1, P], f16))
        xt_s = e(nc.sbuf_tensor([P, XSLOTS * G * P], f16))
        s_s = e(nc.sbuf_tensor([P, SSLOTS * G * W], f16))
        vb16 = e(nc.sbuf_tensor([P, G * 320], f16))
        d8 = e(nc.sbuf_tensor([P, G * 160], f16))
        v8 = e(nc.sbuf_tensor([P, G * 160], f16))
        d4 = e(nc.sbuf_tensor([P, G * 80], f16))
        v4 = e(nc.sbuf_tensor([P, G * 80], f16))
        d2 = e(nc.sbuf_tensor([P, G * 40], f16))
        v2 = e(nc.sbuf_tensor([P, G * 40], f16))
        d1 = e(nc.sbuf_tensor([P, G * 20], f16))
        v1 = e(nc.sbuf_tensor([P, G * 20], f16))
        o0all = e(nc.sbuf_tensor([P, PT], f32))
        o1all = e(nc.sbuf_tensor([P, PT], f32))
        ident_s = e(nc.sbuf_tensor([P, P], f32))
        obuf0 = e(nc.sbuf_tensor([P, P], f32))
        obuf1 = e(nc.sbuf_tensor([P, P], f32))
        ps = e(nc.psum_tensor([P, PSLOTS * 1024], f32))
        dma_c = e(nc.semaphore("dma_c"))
        dma_x = [e(nc.semaphore(f"dma_x{k}")) for k in range(XSLOTS)]
        pe_done = e(nc.semaphore("pe_done"))
        act_done = e(nc.semaphore("act_done"))
        dve_done = e(nc.semaphore("dve_done"))
        block = e(nc.Block())
        n_consts = 4 + (2 if with_bias else 0)

        @block.sync
        def _(sp):
            sp.dma_start(out=wmt_s[:, :], in_=wmt[:, :]).then_inc(dma_c, 16)
            sp.dma_start(out=wax_s[:, :], in_=wax[:, :]).then_inc(dma_c, 16)
            sp.dma_start(out=wbx_s[:, :], in_=wbx[:, :]).then_inc(dma_c, 16)
            sp.dma_start(out=ident_s[:, :], in_=ident[:, :]).then_inc(dma_c, 16)
            if with_bias:
                sp.dma_start(out=brow_s[:, :], in_=brow[:, :]).then_inc(dma_c, 16)
                sp.dma_start(out=ones_s[:, :], in_=ones[:, :]).then_inc(dma_c, 16)
            # prefetch first XSLOTS supertiles of x
            for st in range(min(XSLOTS, NST)):
                sl = st % XSLOTS
                sp.dma_start(
                    out=xt_s[:, sl * G * P:(sl + 1) * G * P],
                    in_=xt[:, st * G * P:(st + 1) * G * P],
                ).then_inc(dma_x[sl], 16)
            for st in range(XSLOTS, NST):
                # prefetch xt for st (slot reused from st-XSLOTS; PE done
                # with it once pe_done reaches 8*(st-XSLOTS+1))
                sl = st % XSLOTS
                sp.wait_ge(pe_done, 8 * (st - XSLOTS + 1))
                sp.dma_start(
                    out=xt_s[:, sl * G * P:(sl + 1) * G * P],
                    in_=xt[:, st * G * P:(st + 1) * G * P],
                ).then_inc(dma_x[sl], 16)
            # tail: store the two transposed output columns
            sp.wait_ge(act_done, 8 * NST + 1)
            sp.dma_start(out=out[0].rearrange("(p x) -> p x", p=P),
                         in_=obuf0[:, :]).then_inc(dma_c, 16)
            sp.dma_start(out=out[1].rearrange("(p x) -> p x", p=P),
                         in_=obuf1[:, :]).then_inc(dma_c, 16)

        @block.tensor
        def _(pe):
            pe.wait_ge(dma_c, 16 * n_consts)
            for st in range(NST):
                xsl = st % XSLOTS
                pe.wait_ge(dma_x[xsl], 16 * (st // XSLOTS + 1))
                for g in range(G):
                    i = st * G + g          # global ptile index
                    psl = i % PSLOTS
                    if i >= PSLOTS:
                        # psum slot reuse: sigmoid of ptile i-PSLOTS done
                        pe.wait_ge(act_done, i - PSLOTS + 1)
                    lhsT = xt_s[:, (xsl * G + g) * P:(xsl * G + g + 1) * P]
                    o = psl * 1024
                    nc.tensor.matmul(ps[:, o:o + 512], lhsT, wmt_s[:, 0:512],
                                     start=True, stop=not with_bias)
                    mm2 = nc.tensor.matmul(ps[:, o + 512:o + 620], lhsT,
                                           wmt_s[:, 512:620],
                                           start=True, stop=not with_bias)
                    if with_bias:
                        # accumulate bias row via K=1 rank-1 matmul
                        nc.tensor.matmul(ps[:, o:o + 512], ones_s[:, :],
                                         brow_s[0:1, 0:512],
                                         start=False, stop=True)
                        mm2 = nc.tensor.matmul(ps[:, o + 512:o + 620],
                                               ones_s[:, :],
                                               brow_s[0:1, 512:620],
                                               start=False, stop=True)
                    mm2.then_inc(pe_done, 1)
            # tail: transpose the per-ptile output columns into row-major
            pe.wait_ge(dve_done, NST)
            nc.tensor.transpose(ps[:, 0:P], o0all[:, :], ident_s[:, :])
            nc.tensor.transpose(ps[:, P:2 * P], o1all[:, :],
                                ident_s[:, :]).then_inc(pe_done, 1)

        @block.scalar
        def _(act):
            act.wait_ge(dma_c, 16 * n_consts)
            # warm-up activations: force the sigmoid spline-table load to
            # complete before the first real sigmoid (the table-load DMA
            # races the first ACTIVATE otherwise -> slightly-wrong values)
            nc.scalar.activation(s_s[:, 0:G * 320], wax_s[:, :],
                                 mybir.ActivationFunctionType.Sigmoid)
            nc.scalar.activation(s_s[:, 0:G * 320], wax_s[:, :],
                                 mybir.ActivationFunctionType.Sigmoid)
            act.drain()
            for st in range(NST):
                ssl = st % SSLOTS
                if st >= SSLOTS:
                    # s-slot reuse: DVE finished supertile st-SSLOTS
                    act.wait_ge(dve_done, st - SSLOTS + 1)
                for g in range(G):
                    i = st * G + g
                    psl = i % PSLOTS
                    act.wait_ge(pe_done, i + 1)
                    o = psl * 1024
                    nc.scalar.activation(
                        s_s[:, (ssl * G + g) * W:(ssl * G + g + 1) * W],
                        ps[:, o:o + 620],
                        mybir.ActivationFunctionType.Sigmoid,
                    ).then_inc(act_done, 1)
            act.wait_ge(pe_done, 8 * NST + 1)
            nc.scalar.copy(obuf0[:, :], ps[:, 0:P])
            nc.scalar.copy(obuf1[:, :], ps[:, P:2 * P]).then_inc(act_done, 1)

        @block.vector
        def _(dve):
            wax_v = wax_s.rearrange("p (g x) -> p g x", g=G)
            wbx_v = wbx_s.rearrange("p (g x) -> p g x", g=G)
            for st in range(NST):
                ssl = st % SSLOTS
                dve.wait_ge(act_done, 8 * (st + 1))
                s_v = s_s[:, ssl * G * W:(ssl + 1) * G * W].rearrange(
                    "p (g x) -> p g x", g=G)

                def lvl(ell, width):
                    o = LVL_OFF[ell] * T
                    return s_v[:, :, o:o + width]

                vb16_v = vb16.rearrange("p (g x) -> p g x", g=G)
                nc.vector.tensor_mul(vb16_v, lvl(4, 320), wbx_v)
                nc.vector.tensor_add(vb16_v, vb16_v, wax_v)

                vcur = vb16
                for ell, half, d_t, v_t in ((3, 160, d8, v8), (2, 80, d4, v4),
                                            (1, 40, d2, v2), (0, 20, d1, v1)):
                    vc_v = vcur.rearrange("p (g x) -> p g x", g=G)
                    d_v = d_t.rearrange("p (g x) -> p g x", g=G)
                    nc.vector.tensor_sub(d_v, vc_v[:, :, half:2 * half],
                                         vc_v[:, :, 0:half])
                    nc.vector.tensor_mul(d_v, lvl(ell, half), d_v)
                    v_v = v_t.rearrange("p (g x) -> p g x", g=G)
                    nc.vector.tensor_add(v_v, vc_v[:, :, 0:half], d_v)
                    vcur = v_t

                o0_sl = o0all[:, st * G:(st + 1) * G]
                nc.vector.tensor_reduce(
                    o0_sl.rearrange("p (g c) -> p g c", c=1),
                    v1.rearrange("p (g x) -> p g x", g=G),
                    mybir.AxisListType.X, mybir.AluOpType.add)
                # the reduce's accumulator writeback is async; drain before
                # reading o0 (intra-DVE RAW) and before signalling consumers
                dve.drain()
                nc.vector.tensor_scalar(
                    o1all[:, st * G:(st + 1) * G], o0_sl, 1.0, -1.0,
                    mybir.AluOpType.subtract, mybir.AluOpType.mult)
                dve.drain().then_inc(dve_done, 1)

    return nc


def _host_prep(x, split_weights, split_bias, leaf_logits, tree_weights,
               feature_masks):
    f32 = np.float32
    Wm = split_weights.astype(f32) * feature_masks.astype(f32)[:, None, :]

    cols_t = np.empty(W, dtype=np.int64)
    cols_n = np.empty(W, dtype=np.int64)
    i = 0
    for ell in range(DEPTH):
        for j in range(2 ** ell):
            node = 2 ** ell - 1 + _bitrev(j, ell)
            for t in range(T):
                cols_t[i] = t
                cols_n[i] = node
                i += 1
    WmT = np.ascontiguousarray(Wm[cols_t, cols_n, :].T)       # [F, 620]
    bias_row = split_bias.astype(f32)[cols_t, cols_n]          # [620]

    ll = leaf_logits.astype(f32)
    e = np.exp(ll - ll.max(axis=-1, keepdims=True))
    lcp = e / e.sum(axis=-1, keepdims=True)
    tw = tree_weights.astype(f32)
    e2 = np.exp(tw - tw.max())
    w = e2 / e2.sum()

    wA = np.empty((16, T), dtype=f32)
    wB = np.empty((16, T), dtype=f32)
    for idx in range(16):
        m4 = _bitrev(idx, 4)
        wA[idx] = w * lcp[:, 2 * m4, 0]
        wB[idx] = w * (lcp[:, 2 * m4 + 1, 0] - lcp[:, 2 * m4, 0])

    xt_full = np.ascontiguousarray(x.astype(f32).T.astype(np.float16))
    wmt_h = WmT.astype(np.float16)
    wax_h = np.tile(np.broadcast_to(wA.reshape(1, 320), (P, 320)),
                    (1, G)).astype(np.float16)
    wbx_h = np.tile(np.broadcast_to(wB.reshape(1, 320), (P, 320)),
                    (1, G)).astype(np.float16)

    with_bias = bool(np.any(split_bias))
    brow_h = None
    if with_bias:
        brow_h = np.broadcast_to(bias_row.reshape(1, W), (P, W)).astype(
            np.float16).copy()
    return xt_full, wmt_h, wax_h, wbx_h, brow_h, with_bias


def kernel(**inputs):
    from concourse.bass_utils import run_bass_kernel_spmd

    x = np.asarray(inputs["x"])
    xt_full, wmt_h, wax_h, wbx_h, brow_h, with_bias = _host_prep(
        x, np.asarray(inputs["split_weights"]), np.asarray(inputs["split_bias"]),
        np.asarray(inputs["leaf_logits"]), np.asarray(inputs["tree_weights"]),
        np.asarray(inputs["feature_masks"]))

    key = ("prog", with_bias)
    if key not in _prog_cache:
        _prog_cache[key] = _build_program(with_bias)
    nc = _prog_cache[key]

    in_maps = []
    for c in range(N_CORES):
        m = {
            "xt": np.ascontiguousarray(xt_full[:, c * BPC:(c + 1) * BPC]),
            "wmt": wmt_h,
            "wax": wax_h,
            "wbx": wbx_h,
            "ident": np.eye(P, dtype=np.float32),
        }
        if with_bias:
            m["brow"] = brow_h
            m["ones"] = np.ones((1, P), dtype=np.float16)
        in_maps.append(m)

    global _last_in_maps
    _last_in_maps = in_maps
    res = run_bass_kernel_spmd(nc, in_maps, list(range(N_CORES)))
    full = np.empty((B, 2), dtype=np.float32)
    for c in range(N_CORES):
        oc = res.results[c]["out"]          # [2, BPC]
        full[c * BPC:(c + 1) * BPC, 0] = oc[0]
        full[c * BPC:(c + 1) * BPC, 1] = oc[1]
    return full
